# revision 70
# baseline (speedup 1.0000x reference)
"""Trainium2 Bass kernel for nn_Adapter_SelfParam_CrossNonParam.

Bottleneck adapter: down-proj(1024->256)+exact GELU, self-attention over
the first 200 prompt tokens (4 heads), parameter-free cross-attention
prompt->tokens, concat, up-proj(256->1024), gate.

Sharding: data-parallel over batch B=64 across 8 NeuronCores (8 items
each); all weights replicated. No collectives.

V9: two-phase schedule split by scalar activation TABLE (gelu vs exp) --
    only 2 ACT_TABLE_LOADs total:
    - Phase A: x loads (sync ring), all 8 down-projs (gelu) + qkv pairs
      (identity, same table). PE saturated by down GEMMs; evac acts
      ordered so PSUM slot WAR never stalls the next pass.
    - Phase B: attention chains FOUR at a time (two persistent shared
      psmA PSUM banks, 4 tokN banks, chain transients split over the
      ch + dA rotations) woven with up-proj half-tile fillers
      (copy/exp table). Keeps the PE p-state ramp warm.
    DMA rings: wpk on scalar, x on sync, stores alternate gpsimd/sync
    (sync is free in Phase B).
    PSUM: one 4-deep [128,512] "dA" rotation shared by Phase-A down
    passes and Phase-B up half-tiles (phases don't overlap), ch 2,
    psmA 2 = 8 banks. Crossout scale-copy on DVE to unload Scalar.
"""
import sys

sys.path.insert(0, "/opt/trn_rl_repo")

import numpy as np
import ml_dtypes
from collections import deque
from contextlib import ExitStack

import concourse.bass as bass
import concourse.tile as tile
from concourse import bacc, mybir
from concourse.bass_utils import run_bass_kernel_spmd

F32 = mybir.dt.float32
BF16 = mybir.dt.bfloat16
AF = mybir.ActivationFunctionType
ADD = mybir.AluOpType.add

B, NTOK, C = 64, 1224, 1024
E, P, T = 256, 200, 1024
NH, HD = 4, 64
NCORES, BL = 8, 8           # cores, batch per core
ATT_SCALE = 1.0 / np.sqrt(HD)   # folded into q weights host-side
CROSS_SCALE = float(E) ** -0.5  # folded into cross-softmax exp scale

# prompt chunks (rows of the 200-token prompt)
PCH = [(0, 128), (128, 72)]


def build_nc():
    nc = bacc.Bacc("TRN2", target_bir_lowering=False, debug=False,
                   num_devices=NCORES)

    x_d = nc.dram_tensor("xb", [BL, 8, 128, NTOK], BF16,
                         kind="ExternalInput").ap()
    wpk_d = nc.dram_tensor("wpk", [128, 6272], BF16, kind="ExternalInput").ap()
    dbias_d = nc.dram_tensor("dbias", [128, 2], F32, kind="ExternalInput").ap()
    qkvb_d = nc.dram_tensor("qkvb", [128, 6], F32, kind="ExternalInput").ap()
    opb_d = nc.dram_tensor("opb", [128, 2], F32, kind="ExternalInput").ap()
    vbr_d = nc.dram_tensor("vbrow", [1, 256], F32, kind="ExternalInput").ap()
    out_d = nc.dram_tensor("out", [BL, NTOK, C], BF16,
                           kind="ExternalOutput").ap()

    with tile.TileContext(nc) as tc, ExitStack() as ctx:
        wp = ctx.enter_context(tc.tile_pool(name="wts", bufs=1))
        sb1 = ctx.enter_context(tc.tile_pool(name="sb1", bufs=1))
        sbr = ctx.enter_context(tc.tile_pool(name="sbr", bufs=2))
        sbx = ctx.enter_context(tc.tile_pool(name="sbx", bufs=3))
        pout = ctx.enter_context(tc.tile_pool(name="pout", bufs=6))
        psD = ctx.enter_context(tc.tile_pool(name="psD", bufs=2, space="PSUM"))
        psU = ctx.enter_context(tc.tile_pool(name="psU", bufs=1, space="PSUM"))
        psC = ctx.enter_context(tc.tile_pool(name="psC", bufs=2, space="PSUM"))
        psA = ctx.enter_context(tc.tile_pool(name="psA", bufs=1, space="PSUM"))

        # ---- resident weights: packed bf16 load on the SCALAR ring ----
        # (scalar engine has no work for the first ~10us; keeps sync free
        # for x loads)
        wpk = wp.tile([128, 6272], BF16, tag="wpk")
        for c0, c1 in ((0, 1024), (1024, 2048), (2048, 4096), (4096, 6272)):
            nc.scalar.dma_start(wpk[:, c0:c1], wpk_d[:, c0:c1])
        # small f32 consts ride the gpsimd ring in parallel
        dbias = wp.tile([128, 2], F32, tag="dbias")
        nc.gpsimd.dma_start(dbias[:], dbias_d[:])
        qkvb = wp.tile([128, 6], F32, tag="qkvb")
        nc.gpsimd.dma_start(qkvb[:], qkvb_d[:])
        opb = wp.tile([128, 2], F32, tag="opb")
        nc.gpsimd.dma_start(opb[:], opb_d[:])
        vbrow = wp.tile([1, 4, 64], F32, tag="vbrow")
        nc.gpsimd.dma_start(vbrow[:1], vbr_d[:1])

        dwT = wpk[:, 0:2048]
        ipWT = [wpk[:, 2048:2816], wpk[:, 2816:3584]]
        opWT = [wpk[:, 3584:3840], wpk[:, 3840:4096]]
        upWT = [wpk[:, 4096:5120], wpk[:, 5120:6144]]
        idB = wpk[:, 6144:6272]

        vbB = wp.tile([128, 4, 64], F32, tag="vbB")
        nc.gpsimd.partition_broadcast(vbB[:], vbrow[:1])
        # out_proj bias replicated along free dim: [128, m, 200]
        opbB = wp.tile([128, 2, P], F32, tag="opbB")
        nc.vector.memset(opbB[:], 0.0)
        for m in range(2):
            nc.vector.tensor_scalar_add(opbB[:, m], opbB[:, m],
                                        opb[:, m:m + 1])
        # persistent tok-natural tiles (+ones col), 4 banks over b
        tokN = [wp.tile([128, 8, 257], BF16, tag=f"tokN{par}",
                        name=f"tokN{par}") for par in range(4)]
        for par in range(4):
            nc.vector.memset(tokN[par][:, :, 256:257], 1.0)
        # two persistent PSUM banks, each hosting psmA regions of 2 chains
        psmAH = [psA.tile([128, 2, 2, 200], BF16, tag=f"psA{i}",
                          name=f"psmAH{i}") for i in range(2)]

        # prompt^T for all 8 batch items: [256 feat, 8*200]
        promT = [wp.tile([128, BL * P], BF16, tag=f"promT{m}",
                         name=f"promT{m}") for m in range(2)]

        def transpose(out_ap, in_ap):
            pw = in_ap.partition_size()
            bp = in_ap.base_partition()
            nc.tensor.transpose(out_ap, in_ap, idB[bp:bp + pw, bp:bp + pw])

        # ------------- x loads: all on the sync ring -------------
        xtiles = {}

        def emit_loads_h(b, h, split=False):
            if h == 0:
                xtiles[b] = [sbx.tile([128, NTOK], BF16, tag=f"xT{ci}",
                                      name=f"xT{b}_{ci}") for ci in range(8)]
            ts = xtiles[b]
            for ci in range(4 * h, 4 * h + 4):
                if split:
                    # partition-split: half the descriptors per queue ->
                    # full tile lands in ~half the time
                    nc.sync.dma_start(ts[ci][0:64, :], x_d[b, ci, 0:64, :])
                    nc.sync.dma_start(ts[ci][64:128, :],
                                      x_d[b, ci, 64:128, :])
                else:
                    nc.sync.dma_start(ts[ci][:], x_d[b, ci])

        # ------------- down-proj: 3 sequential passes (dA bufs=2) -------
        tokT = {}   # (b, m) -> [128, 1024] tok^T e-chunk

        def emit_down_m(b, m):
            xT = xtiles[b]
            tokTm = sb1.tile([128, T], BF16, tag=f"tokT{b}_{m}",
                             name=f"tokT{b}_{m}")
            tokT[(b, m)] = tokTm
            db = dbias[:, m:m + 1]

            def w(ci):
                return dwT[:, ci * 256 + m * 128:ci * 256 + (m + 1) * 128]

            pA1 = psD.tile([128, 512], F32, tag="dA")
            for ci in range(8):
                nc.tensor.matmul(pA1[:], w(ci), xT[ci][:, 0:512],
                                 start=(ci == 0), stop=(ci == 7))
            # evac A1 early (both readers emitted before pB reuses its slot)
            nc.scalar.activation(tokTm[:, 0:312], pA1[:, P:512],
                                 AF.Gelu, bias=db)
            pA2 = psD.tile([128, 512], F32, tag="dA")
            for ci in range(8):
                nc.tensor.matmul(pA2[:], w(ci), xT[ci][:, 512:1024],
                                 start=(ci == 0), stop=(ci == 7))
            nc.scalar.activation(promT[m][:, b * P:(b + 1) * P],
                                 pA1[:, 0:P], AF.Gelu, bias=db)
            nc.scalar.activation(tokTm[:, 312:824], pA2[:],
                                 AF.Gelu, bias=db)
            pB = psD.tile([128, 512], F32, tag="dA")
            for ci in range(8):
                nc.tensor.matmul(pB[:, 0:200], w(ci), xT[ci][:, 1024:NTOK],
                                 start=(ci == 0), stop=(ci == 7))
            if m == 1:
                del xtiles[b]
            nc.scalar.activation(tokTm[:, 824:1024], pB[:, 0:200],
                                 AF.Gelu, bias=db)

        # ---- qkv (q,k feature-major; v natural) per pair (bb, bb+1) ----
        qkvTd = {}
        vN = {}

        def emit_qkv_pair(bb):
            qts = [sbr.tile([128, 2 * P], BF16, tag=f"qkvT{bb}_{mi}",
                            name=f"qkvT{bb}_{mi}", bufs=1) for mi in range(4)]
            for mi in range(4):
                pq = psC.tile([128, 400], F32, tag="ch")
                for ki in range(2):
                    nc.tensor.matmul(
                        pq[:, :400],
                        ipWT[ki][:, mi * 128:(mi + 1) * 128],
                        promT[ki][:, bb * P:(bb + 2) * P],
                        start=(ki == 0), stop=(ki == 1))
                nc.scalar.activation(qts[mi][:], pq[:, :400], AF.Identity,
                                     bias=qkvb[:, mi:mi + 1])
            qkvTd[bb] = qts
            for bv in (bb, bb + 1):
                for pc, (p0, pw) in enumerate(PCH):
                    pv = psC.tile([128, 4, 64], F32, tag="ch")
                    for ki in range(2):
                        nc.tensor.matmul(
                            pv[:pw],
                            promT[ki][:, bv * P + p0:bv * P + p0 + pw],
                            ipWT[ki][:, 512:768],
                            start=(ki == 0), stop=(ki == 1))
                    vt = sb1.tile([128, 4, 65], BF16, tag=f"vN{bv}_{pc}",
                                  name=f"vN{bv}_{pc}")
                    nc.vector.tensor_add(vt[:pw, :, 0:64], pv[:pw], vbB[:pw])
                    nc.vector.memset(vt[:pw, :, 64:65], 1.0)
                    vN[(bv, pc)] = vt

        # ----------------- up-proj half-tiles (fillers) -----------------
        outTd = {}
        store_ring = [0]
        in_phase_a = [True]

        def up_tile(bloc, src, off, mw, orow):
            """One full up-proj tile: [mw, 1024], single-op evacuation
            from a contiguous 2-bank PSUM slot (half the evac queue
            slots of the previous half-tile scheme)."""
            outT = pout.tile([128, C], BF16, tag="outT",
                             name=f"outT{bloc}_{orow}")
            pU = psU.tile([128, 1024], F32, tag="uB")
            for half in range(2):
                for ki in range(2):
                    lh = (src[:, ki, off:off + mw] if src is not None
                          else tokT[(bloc, ki)][:, off:off + mw])
                    nc.tensor.matmul(
                        pU[:mw, half * 512:(half + 1) * 512], lh,
                        upWT[ki][:, half * 512:(half + 1) * 512],
                        start=(ki == 0), stop=(ki == 1))
            if store_ring[0] % 2 == 0:
                nc.scalar.activation(outT[:mw, :], pU[:mw, :],
                                     AF.Copy, bias=0.0)
            else:
                nc.vector.tensor_copy(outT[:mw, :], pU[:mw, :])
            # store: gpsimd-only while Phase A's x loads own the sync
            # ring; alternate rings in Phase B.
            if in_phase_a[0] or store_ring[0] % 2 == 0:
                eng = nc.gpsimd
            else:
                eng = nc.sync
            store_ring[0] += 1
            eng.dma_start(out_d[bloc, orow:orow + mw, :], outT[:mw, :])

        fillers = deque()        # up-proj tok half-tiles
        prio_fillers = deque()   # prompt-row halves (consume promptly)

        def push_up_tok(b):
            for tt in range(8):
                fillers.append((b, None, tt * 128, 128, P + tt * 128))

        def fill(n):
            for _ in range(n):
                if prio_fillers:
                    up_tile(*prio_fillers.popleft())
                elif fillers:
                    up_tile(*fillers.popleft())
                else:
                    return

        # --------------------- attention chain ---------------------
        def chain(b):
            par = b % 4
            tokNb = tokN[par]
            qkvT = qkvTd[b - b % 2]
            boff2 = (b % 2) * P

            def emit_ST(mi, hh):
                kvt = qkvT[2 + mi]
                qvt = qkvT[mi]
                hoff = hh * 64
                pss = psC.tile([128, 400], F32, tag="ch")
                for kc, (k0, kw) in enumerate(PCH):
                    nc.tensor.matmul(
                        pss[:kw, kc * 200:kc * 200 + P],
                        kvt[hoff:hoff + 64, boff2 + k0:boff2 + k0 + kw],
                        qvt[hoff:hoff + 64, boff2:boff2 + P],
                        start=True, stop=True)
                pet = sbr.tile([128, 400], BF16, tag="PeT", bufs=8)
                nc.scalar.activation(pet[:], pss[:], AF.Exp, bias=0.0)
                return pet

            # merged psmA: [128 (hh*64), mi, q]; region of a shared bank
            psmA = psmAH[par % 2][:, par // 2]
            saIn = sbr.tile([128, 2, P], BF16, tag="saIn", name=f"saIn{b}",
                            bufs=4)

            def emit_attnV2(mi, pets):
                """Both heads of E-half mi in one atomic step (one PSUM
                alloc -- keeps the shared ch rotation order-safe)."""
                poq = psC.tile([128, 4, 65], F32, tag="ch", name=f"poq{mi}")
                for hh in range(2):
                    u = 2 * mi + hh
                    for qc, (q0, qw) in enumerate(PCH):
                        for kc, (k0, kw) in enumerate(PCH):
                            nc.tensor.matmul(
                                poq[:qw, 2 * hh + qc, :],
                                pets[hh][:kw,
                                         kc * 200 + q0:kc * 200 + q0 + qw],
                                vN[(b, kc)][:kw, u, :],
                                start=(kc == 0), stop=(kc == 1))
                rq = sbr.tile([128, 4, 1], F32, tag="rq", bufs=8)
                nc.vector.reciprocal(rq[:], poq[:, :, 64:65])
                siq = sbr.tile([128, 4, 64], BF16, tag="siq", bufs=8)
                for hh in range(2):
                    for qc, (q0, qw) in enumerate(PCH):
                        nc.vector.tensor_scalar_mul(
                            siq[:qw, 2 * hh + qc, :],
                            poq[:qw, 2 * hh + qc, 0:64],
                            rq[:qw, 2 * hh + qc, :])
                        transpose(
                            psmA[hh * 64:(hh + 1) * 64, mi, q0:q0 + qw],
                            siq[:qw, 2 * hh + qc, :])

            pet00 = emit_ST(0, 0)
            yield
            pet01 = emit_ST(0, 1)
            yield
            emit_attnV2(0, (pet00, pet01))
            yield
            pet10 = emit_ST(1, 0)
            yield
            pet11 = emit_ST(1, 1)
            yield
            emit_attnV2(1, (pet10, pet11))
            yield
            nc.vector.tensor_copy(saIn[:], psmA[:])

            # out_proj (dA rotation: shares the 4-deep bulk rotation
            # with up-proj half-tiles)
            pm2 = psD.tile([128, 2, P], F32, tag="dA")
            for m in range(2):
                for ki in range(2):
                    nc.tensor.matmul(pm2[:, m, :],
                                     opWT[ki][:, m * 128:(m + 1) * 128],
                                     saIn[:, ki, :],
                                     start=(ki == 0), stop=(ki == 1))
            saT = sbr.tile([128, 2, P], BF16, tag="saT", bufs=4)
            nc.vector.tensor_add(saT[:], pm2[:], opbB[:])
            yield

            # tokN transposes (tokT -> token-major), interleaved w/
            # logits; paired: 8 transposes + ONE DVE copy per 2KB PSUM
            # tile (halves the chain's DVE copy count)
            def emit_tokN2(q):
                psm = psD.tile([128, 4, 256], BF16, tag="dA")
                for j in range(4):
                    tt = 4 * q + j
                    for m in range(2):
                        transpose(psm[:, j, m * 128:(m + 1) * 128],
                                  tokT[(b, m)][:, tt * 128:(tt + 1) * 128])
                nc.vector.tensor_copy(
                    tokNb[:, 4 * q:4 * q + 4, 0:256], psm[:])

            def emit_logits(j):    # logits+exp for token pair
                plt = psD.tile([128, 2, P], F32, tag="dA")
                for i in range(2):
                    tc_ = 2 * j + i
                    for ki in range(2):
                        nc.tensor.matmul(
                            plt[:, i, :],
                            tokT[(b, ki)][:, tc_ * 128:(tc_ + 1) * 128],
                            saT[:, ki, :],
                            start=(ki == 0), stop=(ki == 1))
                pct = sbr.tile([128, 2, P], BF16, tag=f"PcT{j}",
                               name=f"PcT{j}", bufs=4)
                nc.scalar.activation(pct[:], plt[:], AF.Exp,
                                     bias=0.0, scale=CROSS_SCALE)
                return pct

            pcts = []
            for j in range(4):
                if j % 2 == 0:
                    emit_tokN2(j // 2)
                    yield
                pcts.append(emit_logits(j))
                yield

            poN = []

            def emit_crossout(pc):
                p0, pw = PCH[pc]
                pco = psC.tile([128, 257], F32, tag="ch")
                for tc_ in range(8):
                    nc.tensor.matmul(pco[:pw, :257],
                                     pcts[tc_ // 2][:, tc_ % 2, p0:p0 + pw],
                                     tokNb[:, tc_, :],
                                     start=(tc_ == 0), stop=(tc_ == 7))
                rr = sbr.tile([128, 1], F32, tag="rr", bufs=8)
                nc.vector.reciprocal(rr[:pw], pco[:pw, 256:257])
                pn = sbr.tile([128, E], BF16, tag=f"poN{pc}", name=f"poN{pc}",
                              bufs=4)
                nc.vector.tensor_scalar_mul(pn[:pw], pco[:pw, :E], rr[:pw])
                poN.append(pn)

            emit_crossout(0)
            yield
            emit_crossout(1)
            yield

            # poT: prompt_out feature-major
            psm = psD.tile([128, 2, 256], BF16, tag="dA")
            for mi in range(2):
                for pc, (p0, pw) in enumerate(PCH):
                    transpose(psm[:, mi, p0:p0 + pw],
                              poN[pc][:pw, mi * 128:(mi + 1) * 128])
            poT = sbr.tile([128, 2, P], BF16, tag="poT", bufs=4)
            nc.vector.tensor_copy(poT[:], psm[:, :, 0:200])
            yield
            # prompt-row up tiles -> priority fillers
            prio_fillers.append((b, poT, 0, 128, 0))
            prio_fillers.append((b, poT, 128, 72, 128))

        # ================== PHASE A: loads + down + qkv ==================
        # items 0,1: partition-split loads engage all 16 queues so
        # the first down GEMMs can start ~5us earlier
        emit_loads_h(0, 0, split=True)
        emit_loads_h(0, 1, split=True)
        emit_loads_h(1, 0, split=True)
        emit_loads_h(1, 1, split=True)
        for b in range(BL):
            emit_down_m(b, 0)
            if b + 2 < BL:
                emit_loads_h(b + 2, 0)
            if b >= 2:
                fill(1)   # plug x-DMA wait gaps with up-proj tiles
            emit_down_m(b, 1)
            if b + 2 < BL:
                emit_loads_h(b + 2, 1)
            push_up_tok(b)
            if b % 2 == 1:
                emit_qkv_pair(b - 1)
            if b >= 2:
                fill(1)

        # ================== PHASE B: chains + up fillers =================
        in_phase_a[0] = False
        ycnt = [0]
        for bb in range(0, BL, 4):
            gens = [chain(bb + i) for i in range(4)]
            live = [True] * 4
            while any(live):
                for gi in range(4):
                    if live[gi]:
                        try:
                            next(gens[gi])
                            # cluster fillers: a full up tile (4 back-to-
                            # back 512-MMs) every other yield keeps the
                            # PE p-state ramp warm longer than half-tiles
                            # drip-fed each yield
                            ycnt[0] += 1
                            if ycnt[0] % 2 == 0:
                                fill(1)
                        except StopIteration:
                            live[gi] = False
            fill(2)   # inter-quad rebalance
        fill(len(prio_fillers) + len(fillers))

    nc.compile()
    return nc


_NC = None


def _get_nc():
    global _NC
    if _NC is None:
        _NC = build_nc()
    return _NC


def _prep_consts(down_W, down_b, up_W, up_b, in_proj_W, in_proj_b,
                 out_proj_W, out_proj_b, gate):
    f = np.float32
    down_W = np.asarray(down_W, f)
    in_proj_W = np.asarray(in_proj_W, f).copy()
    in_proj_b = np.asarray(in_proj_b, f).copy()
    vbrow = in_proj_b[2 * E:3 * E].reshape(1, 256).copy()
    in_proj_W[:E] *= ATT_SCALE
    in_proj_b[:E] *= ATT_SCALE
    gate = np.float32(np.asarray(gate))
    dwT = np.ascontiguousarray(
        down_W.T.reshape(8, 128, E).transpose(1, 0, 2).reshape(128, 2048))
    ipwt = np.ascontiguousarray(in_proj_W.T.reshape(2, 128, 768))
    opwt = np.ascontiguousarray(
        np.asarray(out_proj_W, f).T.reshape(2, 128, 256))
    upwt = np.ascontiguousarray(
        (np.asarray(up_W, f) * gate).T.reshape(2, 128, 1024))
    bf = ml_dtypes.bfloat16
    wpk = np.concatenate(
        [dwT, ipwt[0], ipwt[1], opwt[0], opwt[1], upwt[0], upwt[1],
         np.eye(128, dtype=f)], axis=1).astype(bf)
    return {
        "wpk": np.ascontiguousarray(wpk),
        "dbias": np.ascontiguousarray(np.asarray(down_b, f).reshape(2, 128).T),
        "qkvb": np.ascontiguousarray(in_proj_b.reshape(6, 128).T),
        "opb": np.ascontiguousarray(np.asarray(out_proj_b, f).reshape(2, 128).T),
        "vbrow": np.ascontiguousarray(vbrow),
    }


def run_kernel(inputs, trace=False):
    """Build in_maps, run on 8 cores, return (full_output, BassKernelResults)."""
    x = np.asarray(inputs["x"], np.float32)
    # feature-major on host: [B, 8ci, 128, NTOK], each (b, ci) contiguous
    xb = np.ascontiguousarray(
        x.astype(ml_dtypes.bfloat16)
        .reshape(B, NTOK, 8, 128).transpose(0, 2, 3, 1))
    consts = _prep_consts(
        inputs["down_W"], inputs["down_b"], inputs["up_W"], inputs["up_b"],
        inputs["in_proj_W"], inputs["in_proj_b"], inputs["out_proj_W"],
        inputs["out_proj_b"], inputs["gate"])
    in_maps = [dict(xb=xb[c * BL:(c + 1) * BL], **consts)
               for c in range(NCORES)]
    nc = _get_nc()
    res = run_bass_kernel_spmd(nc, in_maps, core_ids=list(range(NCORES)),
                               trace=trace)
    out = np.concatenate(
        [res.results[i]["out"] for i in range(NCORES)], axis=0
    ).astype(np.float32)
    up_b = np.asarray(inputs["up_b"], np.float32)
    gate = np.float32(np.asarray(inputs["gate"]))
    if np.any(up_b):
        out = out + gate * up_b
    return out, res


def kernel(**inputs):
    out, _ = run_kernel(inputs, trace=False)
    return out
